# revision 1
# baseline (speedup 1.0000x reference)
"""nn_ChannelAttention Trainium2 Bass kernel (8-core SPMD, data-parallel over batch).

Input  x:   [8, 64, 32, 128, 128] f32
Output att: [8, 64, 1, 1, 1] f32
  n[s]   = sum_c x[c,s]^2
  r[s]   = 1/sqrt(n[s]) = exp(-0.5*ln n)   (channel norms are ~8, eps clamp is a no-op)
  att[c] = sigmoid( relu( mean_s(x*r) + max_s(x*r) )^2 )

Per-core layout: SBUF partitions = (h, c) with h in {0,1} spatial halves,
free = spatial. Tiles [128, F]; 4-tile batches share one PSUM buffer:
  - channel-sumsq via PE matmul (block [128,32] ones lhsT -> 32-row band per
    tile at bases {0,32,64,96}; 16x redundant but PE cost is N-cycle, M-free)
  - ACT Ln then Exp(scale=-0.5) on the redundant [128,F] buffer -> rsqrt
  - PE matmul broadcasts each tile's [2,F] r-slice to [128,F] PSUM; ACT Copy
    exits it to SBUF fp16
  - DVE tensor_mul + tensor_max chains; spatial sum via PE identity-matmul
    accumulation into a resident PSUM tile
"""

from contextlib import ExitStack

import numpy as np

import concourse.bass as bass
import concourse.mybir as mybir
import concourse.tile as tile

F32 = mybir.dt.float32
FP16 = mybir.dt.float16
AF = mybir.ActivationFunctionType
ALU = mybir.AluOpType

B, C, D, H, W = 8, 64, 32, 128, 128
S = D * H * W            # 524288 spatial positions per sample
N_CORES = 8

# tuning knobs
F_TILE = 1024            # tile free width
TPG = 8                  # tiles per DMA group
SQ_ACT_MOD = 5           # every k-th square on ACT, rest on DVE
EXIT_PSUM_MOD = 0        # every k-th tile multiplies straight out of PSUM


def _build_kernel_body(nc, F=F_TILE, TPG=TPG, comp_dt=FP16, sq_act_mod=SQ_ACT_MOD,
                       dma_split=1, exit_psum_mod=EXIT_PSUM_MOD, repeat=1):
    C_, P = 64, 128
    HALF = S // 2
    MM = 512                       # one PSUM bank of f32
    NT = (S * C_) // (P * F)       # total tiles
    GF = F * TPG
    NG = NT // TPG
    assert NG * GF == HALF and TPG % 4 == 0

    x = nc.dram_tensor("x", [C_, S], F32, kind="ExternalInput")
    y = nc.dram_tensor("att", [C_, 1], F32, kind="ExternalOutput")
    xr = x.ap().rearrange("c (h s) -> h c s", h=2)   # element order (h, c, s)

    with tile.TileContext(nc) as tc, ExitStack() as ctx:
        const_pool = ctx.enter_context(tc.tile_pool(name="const", bufs=1))
        gbuf_pool = ctx.enter_context(tc.tile_pool(name="gbuf", bufs=2))
        sq_pool = ctx.enter_context(tc.tile_pool(name="sq", bufs=8))
        nacc_pool = ctx.enter_context(tc.tile_pool(name="nacc", bufs=1, space="PSUM"))
        sacc_pool = ctx.enter_context(tc.tile_pool(name="sacc", bufs=1, space="PSUM"))
        rbp_pool = ctx.enter_context(tc.tile_pool(name="rbp", bufs=2, space="PSUM"))
        rb_pool = ctx.enter_context(tc.tile_pool(name="rb", bufs=6))
        xn_pool = ctx.enter_context(tc.tile_pool(name="xn", bufs=6))
        lall_pool = ctx.enter_context(tc.tile_pool(name="lall", bufs=3))
        acc_pool = ctx.enter_context(tc.tile_pool(name="acc", bufs=1))
        fin_pool = ctx.enter_context(tc.tile_pool(name="fin", bufs=1))

        # lhsT32[p, m] = 1 iff p//64 == m%2  (out row m = n[h=m%2])
        lhsT32 = const_pool.tile([P, 32], comp_dt)
        nc.vector.memset(lhsT32[:], 0.0)
        lo = lhsT32[0:64, :].rearrange("p (m two) -> p m two", two=2)
        nc.vector.memset(lo[:, :, 0:1], 1.0)
        hi = lhsT32[64:128, :].rearrange("p (m two) -> p m two", two=2)
        nc.vector.memset(hi[:, :, 1:2], 1.0)
        # sel2_all rows 32b+h: ones at cols h*64:(h+1)*64 (placed by DMA --
        # engine ops cannot start at partition 32b+1)
        sel2_all = const_pool.tile([P, P], comp_dt)
        nc.vector.memset(sel2_all[:], 0.0)
        rowpat = const_pool.tile([1, 2 * P], comp_dt)
        nc.vector.memset(rowpat[:], 0.0)
        nc.vector.memset(rowpat[0:1, 0:64], 1.0)
        nc.vector.memset(rowpat[0:1, 192:256], 1.0)
        for b4 in range(4):
            nc.gpsimd.dma_start(sel2_all[32 * b4:32 * b4 + 2, :], rowpat[0:1, :])

        ident = const_pool.tile([P, P], comp_dt)
        ones_t = const_pool.tile([P, P], comp_dt)
        nc.vector.memset(ones_t[:], 1.0)
        nc.gpsimd.affine_select(ident[:], ones_t[:], pattern=[[1, P]], base=0,
                                channel_multiplier=-1, compare_op=ALU.is_equal,
                                fill=0.0)

        sacc = sacc_pool.tile([P, F], F32)
        macc_a0 = acc_pool.tile([P, F], comp_dt)
        macc_b0 = acc_pool.tile([P, F], comp_dt)
        macc_a1 = acc_pool.tile([P, F], comp_dt)
        macc_b1 = acc_pool.tile([P, F], comp_dt)
        maccs = [[macc_a0, macc_b0], [macc_a1, macc_b1]]
        for pair in maccs:
            nc.vector.memset(pair[0][:], -2.0)
            nc.vector.memset(pair[1][:], -2.0)

        sq_ct = 0
        for rep in range(repeat):         # >1 only for timing builds
            for g in range(NG):
                gbuf = gbuf_pool.tile([P, F * TPG], comp_dt)
                step = GF // dma_split
                for d in range(dma_split):
                    nc.gpsimd.dma_start(
                        gbuf[:, d * step:(d + 1) * step],
                        xr[:, :, g * GF + d * step: g * GF + (d + 1) * step])

                for bb in range(TPG // 4):
                    nacc = nacc_pool.tile([P, F], F32)
                    for b in range(4):
                        t = bb * 4 + b
                        x_t = gbuf[:, t * F:(t + 1) * F]
                        sq = sq_pool.tile([P, F], comp_dt, tag="sq")
                        sq_ct += 1
                        if sq_act_mod == 0 or (sq_ct % sq_act_mod == 0):
                            nc.scalar.square(sq[:], x_t)
                        else:
                            nc.vector.tensor_mul(sq[:], x_t, x_t)
                        for m0 in range(0, F, MM):
                            nc.tensor.matmul(
                                nacc[32 * b:32 * b + 32, m0:m0 + MM],
                                lhsT32[:], sq[:, m0:m0 + MM],
                                start=True, stop=True, tile_position=(0, 32 * b))

                    l_all = lall_pool.tile([P, F], F32, tag="lall")
                    nc.scalar.activation(l_all[:], nacc[:], AF.Ln)
                    r_all = lall_pool.tile([P, F], comp_dt, tag="rall")
                    nc.scalar.activation(r_all[:], l_all[:], AF.Exp, scale=-0.5)

                    for b in range(4):
                        t = bb * 4 + b
                        gi = g * TPG + t
                        gl = rep * NT + gi
                        x_t = gbuf[:, t * F:(t + 1) * F]
                        rbp = rbp_pool.tile([P, F], F32, tag="rbp")
                        for m0 in range(0, F, MM):
                            nc.tensor.matmul(
                                rbp[:, m0:m0 + MM],
                                sel2_all[32 * b:32 * b + 2, :],
                                r_all[32 * b:32 * b + 2, m0:m0 + MM],
                                start=True, stop=True, tile_position=(32 * b, 0))

                        xn1 = xn_pool.tile([P, F], comp_dt, tag="xn")
                        if exit_psum_mod and (gi % exit_psum_mod) == 0:
                            nc.vector.tensor_mul(xn1[:], x_t, rbp[:])
                        else:
                            rb = rb_pool.tile([P, F], comp_dt, tag="rb")
                            nc.scalar.activation(rb[:], rbp[:], AF.Copy)
                            nc.vector.tensor_mul(xn1[:], x_t, rb[:])
                        for m0 in range(0, F, MM):
                            nc.tensor.matmul(
                                sacc[:, m0:m0 + MM], ident[:], xn1[:, m0:m0 + MM],
                                start=(gl == 0), stop=(gl == repeat * NT - 1),
                                skip_group_check=True)
                        pair = maccs[(gi // 2) % 2]
                        src, dst = pair[gi % 2], pair[1 - gi % 2]
                        nc.vector.tensor_max(dst[:], src[:], xn1[:])

        # ---- finalize ----
        sum_pc = fin_pool.tile([P, 1], F32)
        s_sb = fin_pool.tile([P, F], F32)
        nc.scalar.activation(s_sb[:], sacc[:], AF.Copy)
        nc.vector.reduce_sum(sum_pc[:], s_sb[:], axis=mybir.AxisListType.X)
        mfin0 = fin_pool.tile([P, F], comp_dt)
        nc.vector.tensor_max(mfin0[:], maccs[0][0][:], maccs[0][1][:])
        mfin1 = fin_pool.tile([P, F], comp_dt)
        nc.vector.tensor_max(mfin1[:], maccs[1][0][:], maccs[1][1][:])
        mfin = fin_pool.tile([P, F], comp_dt)
        nc.vector.tensor_max(mfin[:], mfin0[:], mfin1[:])
        max_pc = fin_pool.tile([P, 1], F32)
        nc.vector.reduce_max(max_pc[:], mfin[:], axis=mybir.AxisListType.X)

        # fold halves (partitions 64:128 -> 0:64) via SBUF->SBUF DMA realign
        hi2 = fin_pool.tile([64, 2], F32)
        nc.gpsimd.dma_start(hi2[:, 0:1], sum_pc[64:128, :])
        nc.gpsimd.dma_start(hi2[:, 1:2], max_pc[64:128, :])
        s64 = fin_pool.tile([64, 1], F32)
        nc.vector.tensor_add(s64[:], sum_pc[0:64, :], hi2[:, 0:1])
        m64 = fin_pool.tile([64, 1], F32)
        nc.vector.tensor_max(m64[:], max_pc[0:64, :], hi2[:, 1:2])
        avg = fin_pool.tile([64, 1], F32)
        nc.vector.tensor_scalar_mul(avg[:], s64[:], 1.0 / (S * repeat))
        o = fin_pool.tile([64, 1], F32)
        nc.vector.tensor_add(o[:], avg[:], m64[:])
        orelu = fin_pool.tile([64, 1], F32)
        nc.vector.tensor_scalar_max(orelu[:], o[:], 0.0)
        o2 = fin_pool.tile([64, 1], F32)
        nc.vector.tensor_mul(o2[:], orelu[:], orelu[:])
        att_s = fin_pool.tile([64, 1], F32)
        nc.scalar.activation(att_s[:], o2[:], AF.Sigmoid)
        nc.gpsimd.dma_start(y.ap(), att_s[:])
    return nc


def _split_multi_waits(nc, max_waits=1):
    """This walrus build encodes at most one sync-wait per CTRL instruction;
    hoist extra waits into single-wait NoOps placed just before."""
    for f in nc.m.functions:
        for bb in f.blocks:
            insts = list(bb.instructions)
            out = []
            changed = False
            for ins in insts:
                si = ins.sync_info
                if si is not None and si.on_wait and len(si.on_wait) > max_waits:
                    waits = list(si.on_wait)
                    for w in waits[:-max_waits]:
                        out.append(mybir.InstNoOp(
                            name=nc.get_next_instruction_name(),
                            sync_info=mybir.SyncInfo(on_wait=[w], on_update=[]),
                            bass_nofuse=True,
                            engine=ins.engine,
                        ))
                    si.on_wait = waits[-max_waits:]
                    ins.sync_info = si
                    changed = True
                out.append(ins)
            if changed:
                bb.instructions = out


def build_nc(repeat=1, **kw):
    nc = bass.Bass("TRN2", target_bir_lowering=False, debug=False,
                   num_devices=N_CORES)
    _build_kernel_body(nc, repeat=repeat, **kw)
    _split_multi_waits(nc)
    return nc


def kernel(x):
    """x: [8, 64, 32, 128, 128] f32 -> att [8, 64, 1, 1, 1] f32."""
    from concourse.bass_utils import run_bass_kernel_spmd

    x = np.ascontiguousarray(np.asarray(x, dtype=np.float32))
    assert x.shape == (B, C, D, H, W)
    nc = build_nc()
    in_maps = [{"x": x[i].reshape(C, S)} for i in range(N_CORES)]
    res = run_bass_kernel_spmd(nc, in_maps, core_ids=list(range(N_CORES)))
    att = np.stack([res.results[i]["att"].reshape(C) for i in range(N_CORES)])
    return att.reshape(B, C, 1, 1, 1).astype(np.float32)
